# revision 58
# baseline (speedup 1.0000x reference)
"""Trainium2 Bass kernel for BlockdiagButterflyLinear.

Computes y = butterfly(x; w1, w2) + bias where
  tmp[b,k,j,y] = sum_i x[b, k*1024+i] * w1[k, j*48+y, i]
  out[b, 4l+j] = sum_{k,y} tmp[b,k,j,y] * w2[j, l, k*48+y] + bias[4l+j]

Sharding: data-parallel over the 8192 token rows across 8 NeuronCores
(1024 tokens/core); the small butterfly factors are replicated.

All device traffic is bf16 (the 2e-2 rel-err budget leaves ~50x margin):
x is cast + transposed on the host so no on-chip transposes are needed,
weights are pre-packed into their exact SBUF layouts, and the output is
written bf16 in j-major feature order and un-permuted/upcast on the host.
Per-core HBM traffic: 8.39 MB x + 8.39 MB out + 3.4 MB weights ~= 56 us
at the 360 B/ns DMA roofline; PE work is 131072 matmul rows ~= 55 us at
bf16 full rate, so the kernel is jointly DMA/PE-limited.

Per-core structure (two 512-token halves, th = 0/1):
  stage 1 (per k, th, ic-half): 2x4 accumulating matmuls contract i with
    stationary w1 [128i, 112jy] (16 zero pad columns so both j's of the
    pair land on legal 0/64 partition bases) and moving x^T [128i,
    512tok] into PSUM [112, 512]; copies split the pair into t2[j][c][th]
    tiles [113, 512] (rows 0:48 = k even, 64:112 = k odd, 112 = ones row
    for the bias, 48:64 zero gap).
  stage 2 (per 128-token block m, j, l-half): 2 accumulating matmuls
    with stationary t2 [113, 128tok] and moving w2 [113, 512l] into
    PSUM [128, 512]; copies downcast into [128, 4096] bf16 out tiles
    (j-major feature order), DMA'd out in per-j [128, 1024] quarters.

Schedule: 27 tiny warm-up matmuls burn the tensor engine's slow-pstate
instruction budget before real data lands; the DMA issue order
interleaves th1 x tiles into the th0 stream so PE (one 1.71 us
half-unit per x half-tile) never starves; stage 2 trails with its out
stores riding just behind the copies. Sim/HW exec: ~66.4 us vs a
~61 us PE-chain floor and 56.8 us DMA-busy floor.
"""

import sys

sys.path.insert(0, "/opt/trn_rl_repo")

from contextlib import ExitStack

import numpy as np
import ml_dtypes

import concourse.bacc as bacc
import concourse.bass as bass
import concourse.mybir as mybir
import concourse.tile as tile
from concourse.bass_utils import run_bass_kernel_spmd

F32 = mybir.dt.float32
BF16 = mybir.dt.bfloat16
NP_BF16 = ml_dtypes.bfloat16

N_CORES = 8
TOK = 1024  # tokens per core
N_FEAT = 4096
K, J, B1 = 4, 4, 48
TH = 512  # token half
R2 = 113  # stage-2 contraction rows: 48 + 16 gap + 48 + ones row
N_WARM = 27  # PE warm-up matmuls (256 rows each), tuned against TimelineSim

_PROGRAM = None


def _build_program() -> bass.Bass:
    nc = bacc.Bacc(None, target_bir_lowering=False)
    xs = nc.declare_dram_parameter("xs", [N_FEAT, TOK], BF16, isOutput=False)
    w1p = nc.declare_dram_parameter("w1p", [K, 128, 8, 224], BF16, isOutput=False)
    w2p = nc.declare_dram_parameter("w2p", [J, R2, 2, 1024], BF16, isOutput=False)
    out = nc.declare_dram_parameter("out", [TOK, N_FEAT], BF16, isOutput=True)

    with ExitStack() as ctx:
        tc = ctx.enter_context(tile.TileContext(nc))
        consts = ctx.enter_context(tc.tile_pool(name="consts", bufs=1))
        wpool = ctx.enter_context(tc.tile_pool(name="wpool", bufs=1))
        xpool = ctx.enter_context(tc.tile_pool(name="xpool", bufs=1))
        opool = ctx.enter_context(tc.tile_pool(name="opool", bufs=4))
        p1pool = ctx.enter_context(tc.tile_pool(name="p1pool", bufs=4, space="PSUM"))
        p2pool = ctx.enter_context(tc.tile_pool(name="p2pool", bufs=4, space="PSUM"))

        # w1 split per ic-half so each stage-1 half-unit only waits on the
        # half of the weight/x stream it actually reads
        w1s = [
            [wpool.tile([128, 4, 224], BF16, name=f"w1s_{k}_{h}") for h in (0, 1)]
            for k in range(K)
        ]
        w2s = [wpool.tile([R2, 2, 1024], BF16, name=f"w2s_{j}") for j in range(J)]
        # t2[j][c][th]: stage-2 lhsT tiles. Rows 32:64 zeroed / 96:113 ones
        # once at startup; stage-1 copies then overwrite 0:48 and 64:112,
        # leaving the 48:64 zero gap and the 112 ones (bias) row.
        t2 = [
            [
                [consts.tile([R2, TH], BF16, name=f"t2_{j}_{c}_{th}") for th in (0, 1)]
                for c in (0, 1)
            ]
            for j in range(J)
        ]
        xk = [
            [
                [
                    xpool.tile([128, 4, TH], BF16, name=f"x_{k}_{th}_{h}")
                    for h in (0, 1)
                ]
                for th in (0, 1)
            ]
            for k in range(K)
        ]

        # PE warm-up: the first ~27 tensor-engine matmuls run at reduced
        # pstate. Real matmuls can't start until ~6.7 us (input DMA stream),
        # so burn the slow instructions on throwaway matmuls first.
        warm = consts.tile([128, 256], BF16, name="warm")
        # warm PSUM target shares p1pool's rotation: its buffer is reused by
        # the second stage-1 unit, which starts only after warm-up ends
        pwarm = p1pool.tile([112, TH], F32, tag="p1", name="pwarm")
        nc.vector.memset(warm[:], 0.0)
        for _ in range(N_WARM):
            nc.tensor.matmul(
                pwarm[:, 0:64], warm[:, 0:112], warm[:, 0:64], start=True, stop=True
            )

        for th in (0, 1):
            for c in (0, 1):
                for j in range(J):
                    nc.gpsimd.memset(t2[j][c][th][32:64, :], 0.0)
                    nc.gpsimd.memset(t2[j][c][th][96:R2, :], 1.0)

        def load_x(k, th, h):
            r0 = k * 1024 + h * 512
            nc.sync.dma_start(
                xk[k][th][h][:],
                xs[r0 : r0 + 512, th * TH : (th + 1) * TH].rearrange(
                    "(ic p) t -> p ic t", p=128
                ),
            )

        def load_w1(k, h, eng=None):
            (eng or nc.sync).dma_start(w1s[k][h][:], w1p[k, :, h * 4 : (h + 1) * 4])

        def load_w2(j, c):
            nc.sync.dma_start(w2s[j][:, c, :], w2p[j, :, c, :])

        # Issue order tuned so PE (full rate after warm-up, one 1.71 us
        # half-unit per x half-tile) never waits on a tile that hasn't
        # landed: th1 x loads interleave into the th0 stream; w2 halves
        # slot in just ahead of when stage 2 reads them.
        load_w1(0, 0, eng=nc.scalar)
        load_x(0, 0, 0)
        load_w1(0, 1)
        load_x(0, 0, 1)
        for k, th in [(1, 0), (0, 1), (2, 0), (1, 1), (3, 0), (2, 1)]:
            for h in (0, 1):
                if th == 0:
                    load_w1(k, h)
                load_x(k, th, h)
        load_x(3, 1, 0)
        load_x(3, 1, 1)
        for j in range(J):
            load_w2(j, 0)
            load_w2(j, 1)

        p1live = {}

        def s1a(k, th):
            # first ic-half of both j-pair accumulations; p1 rows: 0:48 =
            # j_even, 16-row gap (zero lhsT columns), 64:112 = j_odd — copy
            # sources land on the 0/64 partition bases the verifier requires.
            ps = [
                p1pool.tile([112, TH], F32, tag="p1", name=f"p1_{k}_{th}_{jp}")
                for jp in range(2)
            ]
            p1live[(k, th)] = ps
            for jp in range(2):
                for ic in range(4):
                    nc.tensor.matmul(
                        ps[jp][:],
                        w1s[k][0][:, ic, jp * 112 : (jp + 1) * 112],
                        xk[k][th][0][:, ic, :],
                        start=(ic == 0),
                        stop=False,
                    )

        def s1b(k, th):
            c = k // 2
            ps = p1live.pop((k, th))
            for jp in range(2):
                for ic in range(4):
                    nc.tensor.matmul(
                        ps[jp][:],
                        w1s[k][1][:, ic, jp * 112 : (jp + 1) * 112],
                        xk[k][th][1][:, ic, :],
                        start=False,
                        stop=(ic == 3),
                    )
                ja, jb = 2 * jp, 2 * jp + 1
                if k % 2 == 0:
                    # partition-preserving copies go on DVE, shifts on ACT
                    nc.vector.tensor_copy(t2[ja][c][th][0:48, :], ps[jp][0:48, :])
                    nc.scalar.copy(t2[jb][c][th][0:48, :], ps[jp][64:112, :])
                else:
                    nc.scalar.copy(t2[ja][c][th][64:112, :], ps[jp][0:48, :])
                    nc.vector.tensor_copy(
                        t2[jb][c][th][64:112, :], ps[jp][64:112, :]
                    )

        _rr = [0]
        _eng = [nc.vector.tensor_copy, nc.scalar.copy]

        outlive = {}

        def s2(m, th, js=range(J), last=False):
            if (m, th) in outlive:
                outm = outlive.pop((m, th))
            else:
                outm = opool.tile([128, N_FEAT], BF16, tag="outm", name=f"o_{m}_{th}")
                outlive[(m, th)] = outm
            row0 = th * TH + m * 128
            for j in js:
                for lc in range(2):
                    p2 = p2pool.tile([128, TH], F32, tag="p2")
                    for c in range(2):
                        nc.tensor.matmul(
                            p2[:],
                            t2[j][c][th][:, m * 128 : (m + 1) * 128],
                            w2s[j][:, c, lc * 512 : (lc + 1) * 512],
                            start=(c == 0),
                            stop=(c == 1),
                        )
                    dst = outm[:, j * 1024 + lc * 512 : j * 1024 + (lc + 1) * 512]
                    _eng[_rr[0] % 2](dst, p2[:])
                    _rr[0] += 1
                if last:
                    # final unit: per-j quarter stores (eighths for the very
                    # last j) shorten the trailing copy->DMA latency chain
                    if j < J - 1:
                        nc.sync.dma_start(
                            out[row0 : row0 + 128, j * 1024 : (j + 1) * 1024],
                            outm[:, j * 1024 : (j + 1) * 1024],
                        )
                    else:
                        for lc in range(2):
                            c0 = j * 1024 + lc * 512
                            nc.sync.dma_start(
                                out[row0 : row0 + 128, c0 : c0 + 512],
                                outm[:, c0 : c0 + 512],
                            )
            if not last:
                for jq in js:
                    nc.sync.dma_start(
                        out[row0 : row0 + 128, jq * 1024 : (jq + 1) * 1024],
                        outm[:, jq * 1024 : (jq + 1) * 1024],
                    )

        # PE unit order matched to the DMA arrival order above: th1 stage-1
        # half-units fill the gaps while th0's x stream finishes; stage 2
        # runs last with its out DMAs riding behind the copies.
        for k, th in [(0, 0), (1, 0), (0, 1), (2, 0), (1, 1), (3, 0), (2, 1)]:
            s1a(k, th)
            s1b(k, th)
        s1a(3, 1)
        s1b(3, 1)
        for m in range(4):
            s2(m, 0)
        for m in range(4):
            s2(m, 1, last=(m == 3))

    nc.compile()
    nc.finalize()
    return nc


def _get_program() -> bass.Bass:
    global _PROGRAM
    if _PROGRAM is None:
        _PROGRAM = _build_program()
    return _PROGRAM


def _prep_weights(w1, w2, b):
    # w1p[k, p, ic, jp*112 + q] = w1[k, (2jp + (q >= 64))*48 + q%64, ic*128+p]
    # with 16 zero columns at 48:64 of each 112-wide j-pair group.
    w1t = w1.transpose(0, 2, 1)  # (k, i, f)
    w1pad = np.zeros((K, 1024, 224), np.float32)
    for jp in range(2):
        w1pad[:, :, jp * 112 : jp * 112 + 48] = w1t[:, :, jp * 96 : jp * 96 + 48]
        w1pad[:, :, jp * 112 + 64 : jp * 112 + 112] = w1t[
            :, :, jp * 96 + 48 : jp * 96 + 96
        ]
    w1p = np.ascontiguousarray(
        w1pad.reshape(K, 8, 128, 224).transpose(0, 2, 1, 3)
    ).astype(NP_BF16)
    # w2p[j, r, c, l]: rows 0:48 = w2[j, l, 96c+y].T, 64:112 = the k-odd
    # half, 112 = bias (only on the c=1 chunk), gap rows zero.
    w2p = np.zeros((J, R2, 2, 1024), np.float32)
    for j in range(J):
        w2j = w2[j]  # (1024 l, 192 ky)
        for c in range(2):
            w2p[j, 0:48, c, :] = w2j[:, 96 * c : 96 * c + 48].T
            w2p[j, 64:112, c, :] = w2j[:, 96 * c + 48 : 96 * c + 96].T
        w2p[j, 112, 1, :] = b[j::J]  # bias[4l+j]
    return w1p, w2p.astype(NP_BF16)


def kernel(x, w1_bfly, w2_bfly, bias):
    x = np.asarray(x, dtype=np.float32)
    w1 = np.asarray(w1_bfly, dtype=np.float32)
    w2 = np.asarray(w2_bfly, dtype=np.float32)
    b = np.asarray(bias, dtype=np.float32)

    x_shape = x.shape
    xb = np.ascontiguousarray(x).reshape(-1, N_FEAT).astype(NP_BF16)
    w1p, w2p = _prep_weights(w1, w2, b)

    nc = _get_program()
    in_maps = [
        {
            "xs": np.ascontiguousarray(xb[c * TOK : (c + 1) * TOK].T),
            "w1p": w1p,
            "w2p": w2p,
        }
        for c in range(N_CORES)
    ]
    res = run_bass_kernel_spmd(nc, in_maps, core_ids=list(range(N_CORES)))
    outs = [np.asarray(res.results[c]["out"]) for c in range(N_CORES)]
    full = np.concatenate(outs, axis=0)  # (8192, 4096) bf16, j-major feats
    full = (
        full.reshape(-1, J, 1024).transpose(0, 2, 1).reshape(-1, N_FEAT)
    ).astype(np.float32)
    return full.reshape(x_shape[:-1] + (N_FEAT,))
